# revision 10
# baseline (speedup 1.0000x reference)
"""Trainium2 Bass kernel for a binarized (1w1a) BasicBlock:

    out = relu(bn2(conv2(sign(pad(relu(bn1(conv1(sign(pad(x)), sign(w1)))))), sign(w2))) + x)

with 2x3 convs, C=256, B=64, H=W=32, pad = (W: 1 left/right, H: 1 bottom).

Strategy: data-parallel over batch across 8 NeuronCores (8 images/core).
Per core the conv is an implicit GEMM: channels on partitions, each of the
6 kernel taps is a [K=128]x[M=128]x[N=512] matmul accumulated in PSUM over
(2 K-tiles x 6 taps). Inputs are binarized to bf16 (+-1/0 exact), so matmul
accumulation in fp32 PSUM is exact integer arithmetic. BN is folded on host
into per-channel scale/bias; conv1's bn+relu+sign epilogue collapses into a
single DVE tensor_scalar ((psum*inv1) is_gt (-bias1) -> {0,1}); conv2's
epilogue is scalar_tensor_tensor (psum*inv2 + x) followed by a Relu
activation with per-channel bias.
"""

import numpy as np
import ml_dtypes

import concourse.mybir as mybir
import concourse.tile as tile
from concourse import bacc
from concourse.bass_utils import run_bass_kernel_spmd

N_CORES = 8
B, C, H, W = 64, 256, 32, 32
BL = B // N_CORES          # images per core
P = 128
KT = C // P                # channel tiles (contraction / output)
HP, WP = H + 1, W + 2      # padded spatial dims (33, 34)
IMG = HP * WP              # 1122
NPOS = 6                   # 2x3 kernel taps
EPS = 1e-5

F32 = mybir.dt.float32
BF16 = mybir.dt.bfloat16
FP8 = mybir.dt.float8e4

# fp8 DoubleRow variant: shared-pad plane layout. Each padded row is 33 wide
# (32 data + 1 shared zero column that serves as row h's right pad AND row
# h+1's left pad), plus one leading zero and a zero bottom row. Conv output
# (h, w) lands at flat position h*33 + w of the 363-column PSUM chunks.
PITCH = 33
DATA0 = 1                   # leading zero (left pad of row 0)
PLANE = DATA0 + PITCH * PITCH   # 1090 = data extent incl bottom pad row
NJ = 3                      # chunks per image (11 rows each)
CH = 11 * PITCH             # 363
NPAD = 1168                 # >= 2*CH + max tap offset (67) + CH, mult of 16
ROWS_J = (11, 11, 10)       # valid output rows per chunk

VARIANT = "fp8"             # "bf16" | "fp8"

_CACHE = {}


def _build():
    if VARIANT == "fp8":
        return _build_fp8()
    return _build_bf16()


def _build_fp8():
    """fp8e4 DoubleRow variant: both channel tiles contract in one PE pass.

    Activations live as [128, 2, NPAD] fp8 tiles (ko-interleaved padded
    planes); each conv output chunk is a [128, 374] PSUM tile covering 11
    padded rows of one image, accumulated over the 6 taps with one
    DoubleRow matmul per tap.
    """
    if "nc" in _CACHE:
        return _CACHE["nc"]

    nc = bacc.Bacc("TRN2", target_bir_lowering=False, debug=False)

    x_d = nc.dram_tensor("x", [BL, C, H, W], F32, kind="ExternalInput")
    w1_d = nc.dram_tensor("w1t", [P, KT, NPOS, C], FP8, kind="ExternalInput")
    w2_d = nc.dram_tensor("w2t", [P, KT, NPOS, C], FP8, kind="ExternalInput")
    inv1_d = nc.dram_tensor("inv1", [C], F32, kind="ExternalInput")
    nb1_d = nc.dram_tensor("nb1", [C], F32, kind="ExternalInput")
    inv2_d = nc.dram_tensor("inv2", [C], F32, kind="ExternalInput")
    b2_d = nc.dram_tensor("b2", [C], F32, kind="ExternalInput")
    out_d = nc.dram_tensor("out", [BL, C, H, W], F32, kind="ExternalOutput")

    with tile.TileContext(nc) as tc:
        with (
            tc.tile_pool(name="res", bufs=1) as res,
            tc.tile_pool(name="tmp", bufs=4) as tmp,
            tc.tile_pool(name="stg", bufs=4) as stg,
            tc.tile_pool(name="ps", bufs=6, space="PSUM") as ps,
        ):
            xf32 = [[None] * BL for _ in range(KT)]
            xq1 = [None] * BL
            xq2 = [None] * BL

            def pad_memsets(q, eng):
                """Zero only the pad cells: leading zero, shared pad column,
                bottom pad row, tail. Small strided memsets, pinned off the
                Scalar engine so SIGNs aren't delayed."""
                v = q[:, :, DATA0:DATA0 + PITCH * PITCH].rearrange(
                    "c k (h w) -> c k h w", w=PITCH)
                eng.memset(q[:, :, 0:DATA0], 0.0)
                eng.memset(v[:, :, :, W:PITCH], 0.0)
                eng.memset(v[:, :, H:PITCH, 0:W], 0.0)
                eng.memset(q[:, :, PLANE:NPAD], 0.0)

            def interior(q, kt):
                return q[:, kt, DATA0:DATA0 + H * PITCH].rearrange(
                    "c (h w) -> c h w", w=PITCH)[:, :, 0:W]

            def load_b(b):
                q1 = res.tile([P, KT, NPAD], FP8, tag=f"xq1_{b}", name=f"xq1_{b}")
                pad_memsets(q1, nc.gpsimd)
                xq1[b] = q1
                q2 = res.tile([P, KT, NPAD], FP8, tag=f"xq2_{b}", name=f"xq2_{b}")
                pad_memsets(q2, nc.vector)
                xq2[b] = q2
                for kt in range(KT):
                    xt = res.tile([P, H * W], F32, tag=f"xf_{kt}_{b}", name=f"xf_{kt}_{b}")
                    nc.sync.dma_start(
                        xt[:],
                        x_d.ap()[b, kt * P:(kt + 1) * P].rearrange("c h w -> c (h w)"),
                    )
                    xf32[kt][b] = xt
                    nc.scalar.sign(
                        interior(q1, kt),
                        xt.rearrange("c (h w) -> c h w", w=W),
                    )

            load_b(0)

            w1sb = res.tile([P, KT, NPOS, C], FP8, tag="w1q", name="w1q")
            nc.sync.dma_start(w1sb[:], w1_d.ap())

            load_b(1)

            w2sb = res.tile([P, KT, NPOS, C], FP8, tag="w2q", name="w2q")
            nc.sync.dma_start(w2sb[:], w2_d.ap())

            def load_vec(d, nm):
                t = res.tile([P, KT], F32, tag=nm, name=nm)
                nc.sync.dma_start(t[:], d.ap().rearrange("(t p) -> p t", p=P))
                return t

            inv1sb = load_vec(inv1_d, "inv1sb")
            nb1sb = load_vec(nb1_d, "nb1sb")
            inv2sb = load_vec(inv2_d, "inv2sb")
            b2sb = load_vec(b2_d, "b2sb")

            for b in range(2, BL):
                load_b(b)

            def conv_groups(b, mt, wsb, src):
                """6-tap DoubleRow accumulation for the NJ chunks of (b, mt).

                pos-outer / chunk-inner so consecutive matmuls share lhsT.
                Returns the NJ psum tiles.
                """
                pts = [
                    ps.tile([P, CH], F32, tag="ps", name=f"ps_{b}_{mt}_{j}")
                    for j in range(NJ)
                ]
                for pos in range(NPOS):
                    kh, kw = divmod(pos, 3)
                    off = kh * PITCH + kw
                    for j in range(NJ):
                        nc.tensor.matmul(
                            pts[j][:],
                            wsb[:, :, pos, mt * P:(mt + 1) * P],
                            src[:, :, off + j * CH: off + j * CH + CH],
                            start=(pos == 0),
                            stop=(pos == NPOS - 1),
                            perf_mode=mybir.MatmulPerfMode.DoubleRow,
                        )
                return pts

            # ---- conv1 + binarize epilogue ----
            for b in range(BL):
                for mt in range(KT):
                    pts = conv_groups(b, mt, w1sb, xq1[b])
                    q2v = interior(xq2[b], mt)
                    for j in range(NJ):
                        r = ROWS_J[j]
                        nc.vector.tensor_scalar(
                            q2v[:, 11 * j:11 * j + r, :],
                            pts[j].rearrange("c (r w) -> c r w", w=PITCH)[:, 0:r, 0:W],
                            inv1sb[:, mt:mt + 1],
                            nb1sb[:, mt:mt + 1],
                            mybir.AluOpType.mult,
                            mybir.AluOpType.is_gt,
                        )

            # ---- conv2 + bn2 + residual + relu ----
            for b in range(BL):
                for mt in range(KT):
                    pts = conv_groups(b, mt, w2sb, xq2[b])
                    for j in range(NJ):
                        r = ROWS_J[j]
                        n = r * W
                        tt = tmp.tile([P, 11 * W], F32, tag="t2", name=f"t2_{b}_{mt}_{j}")
                        nc.vector.scalar_tensor_tensor(
                            tt[:, 0:n].rearrange("c (r w) -> c r w", w=W),
                            pts[j].rearrange("c (r w) -> c r w", w=PITCH)[:, 0:r, 0:W],
                            inv2sb[:, mt:mt + 1],
                            xf32[mt][b][:, 11 * j * W: 11 * j * W + n]
                                .rearrange("c (r w) -> c r w", w=W),
                            mybir.AluOpType.mult,
                            mybir.AluOpType.add,
                        )
                        ot = stg.tile([P, 11 * W], F32, tag="ot", name=f"ot_{b}_{mt}_{j}")
                        nc.scalar.activation(
                            ot[:, 0:n], tt[:, 0:n],
                            mybir.ActivationFunctionType.Relu,
                            bias=b2sb[:, mt:mt + 1],
                            scale=1.0,
                        )
                        nc.sync.dma_start(
                            out_d.ap()[b, mt * P:(mt + 1) * P]
                                 .rearrange("c h w -> c (h w)")[:, 11 * j * W: 11 * j * W + n],
                            ot[:, 0:n],
                        )

    nc.compile()
    _CACHE["nc"] = nc
    return nc


def _build_bf16():
    """Emit the single-core Tile program (same NEFF runs SPMD on all cores)."""
    if "nc" in _CACHE:
        return _CACHE["nc"]

    nc = bacc.Bacc("TRN2", target_bir_lowering=False, debug=False)

    x_d = nc.dram_tensor("x", [BL, C, H, W], F32, kind="ExternalInput")
    w1_d = nc.dram_tensor("w1t", [KT, P, NPOS, C], BF16, kind="ExternalInput")
    w2_d = nc.dram_tensor("w2t", [KT, P, NPOS, C], BF16, kind="ExternalInput")
    inv1_d = nc.dram_tensor("inv1", [C], F32, kind="ExternalInput")
    nb1_d = nc.dram_tensor("nb1", [C], F32, kind="ExternalInput")
    inv2_d = nc.dram_tensor("inv2", [C], F32, kind="ExternalInput")
    b2_d = nc.dram_tensor("b2", [C], F32, kind="ExternalInput")
    out_d = nc.dram_tensor("out", [BL, C, H, W], F32, kind="ExternalOutput")

    with tile.TileContext(nc) as tc:
        with (
            tc.tile_pool(name="res", bufs=1) as res,
            tc.tile_pool(name="tmp", bufs=4) as tmp,
            tc.tile_pool(name="stg", bufs=4) as stg,
            tc.tile_pool(name="ps", bufs=6, space="PSUM") as ps,
        ):
            # ---- resident activations ----
            # xf32[kt][b]: original x, [128, 1024] f32 (residual + sign source)
            # xpad1/xpad2[kt][b]: padded binarized inputs, [128, 1122] bf16
            xf32 = [[None] * BL for _ in range(KT)]
            xpad1 = [[None] * BL for _ in range(KT)]
            xpad2 = [[None] * BL for _ in range(KT)]
            xp1v = [[None] * BL for _ in range(KT)]
            xp2v = [[None] * BL for _ in range(KT)]

            def load_b(b):
                """DMA + binarize image b (both channel tiles)."""
                for kt in range(KT):
                    xt = res.tile([P, H * W], F32, tag=f"xf_{kt}_{b}", name=f"xf_{kt}_{b}")
                    nc.sync.dma_start(
                        xt[:],
                        x_d.ap()[b, kt * P:(kt + 1) * P].rearrange("c h w -> c (h w)"),
                    )
                    xf32[kt][b] = xt

                    p1 = res.tile([P, IMG], BF16, tag=f"xp1_{kt}_{b}", name=f"xp1_{kt}_{b}")
                    xpad1[kt][b] = p1
                    v1 = p1.rearrange("c (h w) -> c h w", w=WP)
                    xp1v[kt][b] = v1
                    # zero only the padding: col 0, col 33, bottom row interior
                    nc.any.memset(v1[:, :, 0:1], 0.0)
                    nc.any.memset(v1[:, :, WP - 1:WP], 0.0)
                    nc.any.memset(v1[:, HP - 1:HP, 1:W + 1], 0.0)
                    # binarize interior: sign(x) in {-1, +1}
                    nc.scalar.sign(
                        v1[:, 0:H, 1:W + 1],
                        xt.rearrange("c (h w) -> c h w", w=W),
                    )

                    p2 = res.tile([P, IMG], BF16, tag=f"xp2_{kt}_{b}", name=f"xp2_{kt}_{b}")
                    xpad2[kt][b] = p2
                    v2 = p2.rearrange("c (h w) -> c h w", w=WP)
                    xp2v[kt][b] = v2
                    nc.any.memset(v2[:, :, 0:1], 0.0)
                    nc.any.memset(v2[:, :, WP - 1:WP], 0.0)
                    nc.any.memset(v2[:, HP - 1:HP, 1:W + 1], 0.0)

            # image 0 first: it gates the first matmul group
            load_b(0)

            # ---- resident weights / BN vectors ----
            w1sb, w2sb = [], []
            for kt in range(KT):
                t1 = res.tile([P, NPOS, C], BF16, tag=f"w1_{kt}", name=f"w1_{kt}")
                nc.sync.dma_start(t1[:], w1_d.ap()[kt])
                w1sb.append(t1)

            load_b(1)

            for kt in range(KT):
                t2 = res.tile([P, NPOS, C], BF16, tag=f"w2_{kt}", name=f"w2_{kt}")
                nc.sync.dma_start(t2[:], w2_d.ap()[kt])
                w2sb.append(t2)

            def load_vec(d, nm):
                t = res.tile([P, KT], F32, tag=nm, name=nm)
                nc.sync.dma_start(t[:], d.ap().rearrange("(t p) -> p t", p=P))
                return t

            inv1sb = load_vec(inv1_d, "inv1sb")
            nb1sb = load_vec(nb1_d, "nb1sb")
            inv2sb = load_vec(inv2_d, "inv2sb")
            b2sb = load_vec(b2_d, "b2sb")

            for b in range(2, BL):
                load_b(b)

            # ---- conv1: 12-matmul PSUM groups + fused bn/relu/sign epilogue ----
            for b in range(BL):
                for h0 in (0, 1):                 # output rows [16*h0, 16*h0+16)
                    for mt in range(KT):          # output-channel tile
                        pt = ps.tile([P, 512], F32, tag="ps", name=f"ps1_{b}_{h0}_{mt}")
                        first = True
                        for kt in range(KT):
                            for pos in range(NPOS):
                                kh, kw = divmod(pos, 3)
                                nc.tensor.matmul(
                                    pt[:],
                                    w1sb[kt][:, pos, mt * P:(mt + 1) * P],
                                    xp1v[kt][b][:, 16 * h0 + kh:16 * h0 + kh + 16,
                                                kw:kw + W],
                                    start=first,
                                    stop=(kt == KT - 1 and pos == NPOS - 1),
                                )
                                first = False
                        # sign(relu(psum*inv1 + bias1)) = (psum*inv1 > -bias1)
                        nc.vector.tensor_scalar(
                            xp2v[mt][b][:, 16 * h0:16 * h0 + 16, 1:W + 1],
                            pt[:],
                            inv1sb[:, mt:mt + 1],
                            nb1sb[:, mt:mt + 1],
                            mybir.AluOpType.mult,
                            mybir.AluOpType.is_gt,
                        )

            # ---- conv2: same groups + bn2 + residual + relu epilogue ----
            for b in range(BL):
                for h0 in (0, 1):
                    for mt in range(KT):
                        pt = ps.tile([P, 512], F32, tag="ps", name=f"ps2_{b}_{h0}_{mt}")
                        first = True
                        for kt in range(KT):
                            for pos in range(NPOS):
                                kh, kw = divmod(pos, 3)
                                nc.tensor.matmul(
                                    pt[:],
                                    w2sb[kt][:, pos, mt * P:(mt + 1) * P],
                                    xp2v[kt][b][:, 16 * h0 + kh:16 * h0 + kh + 16,
                                                kw:kw + W],
                                    start=first,
                                    stop=(kt == KT - 1 and pos == NPOS - 1),
                                )
                                first = False
                        # t = psum*inv2 + x  (residual), then out = relu(t + beta2')
                        tt = tmp.tile([P, 512], F32, tag="t2", name=f"t2_{b}_{h0}_{mt}")
                        nc.vector.scalar_tensor_tensor(
                            tt[:],
                            pt[:],
                            inv2sb[:, mt:mt + 1],
                            xf32[mt][b][:, 512 * h0:512 * h0 + 512],
                            mybir.AluOpType.mult,
                            mybir.AluOpType.add,
                        )
                        ot = stg.tile([P, 512], F32, tag="ot", name=f"ot_{b}_{h0}_{mt}")
                        nc.scalar.activation(
                            ot[:], tt[:],
                            mybir.ActivationFunctionType.Relu,
                            bias=b2sb[:, mt:mt + 1],
                            scale=1.0,
                        )
                        nc.sync.dma_start(
                            out_d.ap()[b, mt * P:(mt + 1) * P]
                                 .rearrange("c h w -> c (h w)")[:, 512 * h0:512 * h0 + 512],
                            ot[:],
                        )

    nc.compile()
    _CACHE["nc"] = nc
    return nc


def _prep(w1, w2, gamma1, beta1, mean1, var1, gamma2, beta2, mean2, var2):
    """Host-side: fold BN, binarize + lay out weights as lhsT tiles."""
    def fold(gamma, beta, mean, var):
        inv = (gamma.astype(np.float64) / np.sqrt(var.astype(np.float64) + EPS))
        inv = inv.astype(np.float32)
        bias = (beta.astype(np.float32) - mean.astype(np.float32) * inv)
        return inv, bias

    inv1, bias1 = fold(gamma1, beta1, mean1, var1)
    inv2, bias2 = fold(gamma2, beta2, mean2, var2)

    if VARIANT == "fp8":
        def wt(w):
            # [O, I, 2, 3] -> DoubleRow lhsT layout [ci, ko, pos, co]
            s = np.sign(w).astype(np.float32)
            arr = s.transpose(1, 2, 3, 0).reshape(KT, P, NPOS, C).transpose(1, 0, 2, 3)
            return np.ascontiguousarray(arr).astype(mybir.dt.np(FP8))
    else:
        def wt(w):
            # [O, I, 2, 3] -> lhsT layout [kt, ci, pos, co]
            s = np.sign(w).astype(ml_dtypes.bfloat16)
            return np.ascontiguousarray(
                s.transpose(1, 2, 3, 0).reshape(KT, P, NPOS, C)
            )

    return wt(w1), wt(w2), inv1, -bias1, inv2, bias2


def kernel(x, w1, gamma1, beta1, mean1, var1,
           w2, gamma2, beta2, mean2, var2):
    x = np.asarray(x, dtype=np.float32)
    w1t, w2t, inv1, nb1, inv2, b2 = _prep(
        np.asarray(w1), np.asarray(w2),
        np.asarray(gamma1), np.asarray(beta1), np.asarray(mean1), np.asarray(var1),
        np.asarray(gamma2), np.asarray(beta2), np.asarray(mean2), np.asarray(var2),
    )

    nc = _build()
    in_maps = []
    for c in range(N_CORES):
        in_maps.append({
            "x": np.ascontiguousarray(x[c * BL:(c + 1) * BL]),
            "w1t": w1t, "w2t": w2t,
            "inv1": inv1, "nb1": nb1, "inv2": inv2, "b2": b2,
        })

    res = run_bass_kernel_spmd(nc, in_maps, core_ids=list(range(N_CORES)))
    out = np.concatenate([r["out"] for r in res.results], axis=0)
    return out
